# revision 33
# baseline (speedup 1.0000x reference)
"""CLOCs pairwise-IoU association kernel for Trainium2 (8 NeuronCores).

Problem: boxes [N=10000,4], query_boxes [K=500,4] -> dense association tensor
  overlaps     [K*N, 4] f32 : (iou|-10, s3d, s2d|-10, dis) per (k,n) pair
  tensor_index [K*N, 2] i32 : (k, n)
  valid        [K*N]   bool : iw>0 & ih>0

Sharding: N split into 8 contiguous strips of 1250 (one per core); every core
holds all K (padded to 512) query boxes on the partition axis, 4 k-tiles of
128 (128-partition DMAs spread across all 16 SDMA engines; narrower ones
collapse onto 4 and serialize).

Broadcast strategy (all exact):
  - box coords + index row: stride-0 partition-broadcast DMA from HBM (SWDGE)
  - (s3d,dis) pair row: TensorE ones[3,128] @ bf16x3(hi/mid/lo) split rows,
    summed in fp32 PSUM -> exact fp32 broadcast at bf16 matmul speed
  - ua0 = box_area[n] + qbox_area[k]: same matmul with the k-term carried in
    three extra lhsT weight rows (bf16x3 split of qarea) against ones rows.
K-side per-box values ride per-partition scalar operands.

Per k-tile (free dim = 1250 n's):
  DVE : t2=max(bx1,qx1); iw=min(bx2,qx2)-t2; t4=max(by1,qy1);
        ih=min(by2,qy2)-t4; inter=max(iw,0)*relu(ih); ua=ua0-inter; r~=1/ua;
        vd=(inter<=0); iou=inter*r; out0=vd*-10+iou
  ACT : relu(ih); out2=scale*vd+bias (c2 select); valid=sign(inter) u8
  PE  : ua0 tile into PSUM
  POOL: tensor_index even lanes = k (off critical path)
The (s3d,dis) output lanes and index odd lanes are constant across k-tiles:
baked once into ping-pong output tiles at setup, only lanes 0/2 rewritten.
Output DMAs are split into column chunks to keep several in flight.
"""

import numpy as np

N = 10000
K = 500
KP = 512                    # K padded to 4*128 partitions
NCORES = 8
NLOC = N // NCORES          # 1250
KT = 4                      # k-tiles of 128
OV_CH = 5                   # overlaps DMA column chunks per k-tile
IX_CH = 2                   # index DMA column chunks per k-tile

# column ranges inside the bf16 broadcast-source array BC [6, 11250]
_BC_AREA = 0
_BC_S3D = NLOC              # 2*NLOC wide, (s3d, dis) interleaved
_BC_BX1 = 3 * NLOC
_BC_BX2 = 4 * NLOC
_BC_BY1 = 5 * NLOC
_BC_BY2 = 6 * NLOC
_BC_IDX = 7 * NLOC          # 2*NLOC wide, (0, n) interleaved
_BC_W = 9 * NLOC

_kernel_cache = {}
_custom_ops = None


def _register_custom_dve_ops():
    """Register two fused DVE ops (runtime-compiled uop programs):
      IW_RELU_ANT : relu(min(Src0, s0) - max(Src1, s1))  -> interval width
      IOU_NEG_ANT : Src0*Src1 + s0*(Src0 <= 0)           -> iou | sentinel
    """
    global _custom_ops
    if _custom_ops is not None:
        return _custom_ops
    import re
    import concourse.dve_ops as dops
    from concourse.dve_spec import Spec, Src0, Src1, C0, C1, Zero, minn, maxx

    def _ref_iw(in0, in1, s0, s1, imm2):
        return np.maximum(
            np.minimum(in0, s0) - np.maximum(in1, s1), 0.0
        ).astype(np.float32)

    def _ref_c0(in0, in1, s0, s1, imm2):
        return (in0 * in1 + s0 * (in0 <= 0.0)).astype(np.float32)

    iw_op = dops.DveOp(
        "IW_RELU_ANT",
        Spec(body=maxx(minn(Src0, C0) - maxx(Src1, C1), Zero),
             reference=_ref_iw),
        subdim=False, uops_sha={})
    c0_op = dops.DveOp(
        "IOU_NEG_ANT",
        Spec(body=Src0 * Src1 + C0 * (Src0 <= Zero),
             reference=_ref_c0),
        subdim=False, uops_sha={})
    next_row = max(dops._SUB_OPCODE_FOR_NAME.values()) + 1
    for op in (iw_op, c0_op):
        dops.OPS.append(op)
        dops.CUSTOM_DVE_SPECS[op.name] = op.spec
        dops._SUB_OPCODE_FOR_NAME[op.name] = next_row
        next_row += 1
        for ver in ("v3",):
            try:
                op.compile(ver)
            except ValueError as e:
                m = re.search(rf"\({ver}: ([0-9a-f]+)", str(e))
                if not m:
                    raise
                op.uops_sha[ver] = m.group(1)
            op.compile(ver)
    _custom_ops = (iw_op, c0_op)
    return _custom_ops


def _build_kernel(criterion: int):
    import concourse.bacc as bacc
    import concourse.tile as tile
    from concourse import mybir

    iw_op, c0_op = _register_custom_dve_ops()

    f32 = mybir.dt.float32
    bf16 = mybir.dt.bfloat16
    i32 = mybir.dt.int32
    u8 = mybir.dt.uint8
    Alu = mybir.AluOpType
    Act = mybir.ActivationFunctionType

    nc = bacc.Bacc("TRN2", target_bir_lowering=False, debug=False,
                   num_devices=NCORES)

    A = nc.dram_tensor("A", [4, NLOC], f32, kind="ExternalInput").ap()
    BC = nc.dram_tensor("BC", [6, _BC_W], bf16, kind="ExternalInput").ap()
    WUA = nc.dram_tensor("WUA", [6, 128 * (KT + 1)], bf16,
                         kind="ExternalInput").ap()
    Q = nc.dram_tensor("Q", [128, 8 * KT], f32, kind="ExternalInput").ap()

    OV = nc.dram_tensor("OV", [KP, 4 * NLOC], f32, kind="ExternalOutput").ap()
    IX = nc.dram_tensor("IX", [KP, 2 * NLOC], i32, kind="ExternalOutput").ap()
    VA = nc.dram_tensor("VA", [KP, NLOC], u8, kind="ExternalOutput").ap()

    # criterion==-1 subtracts inter from the union denominator
    gamma = 1.0 if criterion == -1 else 0.0

    with tile.TileContext(nc) as tc:
        with (
            tc.tile_pool(name="const", bufs=1) as const,
            tc.tile_pool(name="tmp", bufs=1) as tmpp,
            tc.tile_pool(name="outv", bufs=2) as outv,
            tc.tile_pool(name="ps", bufs=2, space="PSUM") as ps,
        ):
            Q_sb = const.tile([128, 8 * KT], f32)
            BC_sb = const.tile([6, _BC_W], bf16)
            WUA_sb = const.tile([6, 128 * (KT + 1)], bf16)
            nc.sync.dma_start(out=Q_sb, in_=Q)
            nc.sync.dma_start(out=BC_sb, in_=BC)
            nc.sync.dma_start(out=WUA_sb, in_=WUA)
            ONES6 = WUA_sb[:, 128 * KT:128 * (KT + 1)]

            BX1 = const.tile([128, NLOC], f32, tag="BX1")
            BX2 = const.tile([128, NLOC], f32, tag="BX2")
            BY1 = const.tile([128, NLOC], f32, tag="BY1")
            BY2 = const.tile([128, NLOC], f32, tag="BY2")
            # coords gate the first DVE op: replicate them via stride-0 DMA
            # (starts immediately, independent of the PE/ACT prologue)
            for row, dst in ((0, BX1), (1, BX2), (2, BY1), (3, BY2)):
                nc.gpsimd.dma_start(
                    out=dst,
                    in_=A[row:row + 1, :].to_broadcast([128, NLOC]))
            Ts = [const.tile([128, NLOC, 4], f32, tag=f"T{i}", name=f"T{i}")
                  for i in range(2)]
            IDXs = [const.tile([128, NLOC, 2], i32, tag=f"IDX{i}",
                               name=f"IDX{i}") for i in range(2)]

            def bcast(col0, width, write):
                """TensorE-exact broadcast of BC cols [col0, col0+width) to
                128 partitions; `write(psum_chunk, off, cs)` stores it."""
                off = 0
                while off < width:
                    cs = min(512, width - off)
                    pb = ps.tile([128, 512], f32, tag="pb", name="pb")
                    nc.tensor.matmul(pb[:, :cs], ONES6,
                                     BC_sb[:, col0 + off:col0 + off + cs],
                                     start=True, stop=True)
                    write(pb[:, :cs], off, cs)
                    off += cs

            def wr_s3d(pbc, off, cs):
                # chunk cols are (s3d,dis) pairs -> T lanes (1,3)
                for T in Ts:
                    nc.scalar.activation(
                        T[:, off // 2:(off + cs) // 2, 1::2],
                        pbc.rearrange("p (f c) -> p f c", c=2), Act.Copy)
            bcast(_BC_S3D, 2 * NLOC, wr_s3d)

            def wr_idx(pbc, off, cs):
                for idx_t in IDXs:
                    nc.scalar.activation(
                        idx_t.rearrange("p f c -> p (f c)")[:, off:off + cs],
                        pbc, Act.Copy)
            bcast(_BC_IDX, 2 * NLOC, wr_idx)

            # tiles 0/1 index blocks are fully determined at setup: fill the
            # k lane and ship them early so the IX DMAs fill the DMA-idle
            # window before the first overlaps tiles complete
            cwi = 2 * NLOC // IX_CH
            for t in range(2):
                IDX = IDXs[t]
                nc.gpsimd.tensor_scalar(
                    IDX[:, :, 0], IDX[:, :, 1], 0.0,
                    Q_sb[:, 8 * t + 6:8 * t + 7], Alu.mult, Alu.add)
                ixf = IDX.rearrange("p f c -> p (f c)")
                for ci in range(IX_CH):
                    nc.sync.dma_start(
                        out=IX[128 * t:128 * (t + 1),
                               ci * cwi:(ci + 1) * cwi],
                        in_=ixf[:, ci * cwi:(ci + 1) * cwi])

            # --- steady: one pass per k-tile ---
            for t in range(KT):
                T = Ts[t % 2]
                IDX = IDXs[t % 2]

                def q(j, t=t):
                    return Q_sb[:, 8 * t + j:8 * t + j + 1]

                # ua0 = barea + qarea on TensorE (exact, bf16x3 both sides)
                UA0 = ps.tile([128, NLOC], f32, tag="ua", name="ua0p")
                for c0 in range(0, NLOC, 512):
                    cs = min(512, NLOC - c0)
                    nc.tensor.matmul(UA0[:, c0:c0 + cs],
                                     WUA_sb[:, 128 * t:128 * (t + 1)],
                                     BC_sb[:, _BC_AREA + c0:_BC_AREA + c0 + cs],
                                     start=True, stop=True)

                iwp = tmpp.tile([128, NLOC], f32, tag="iwp", bufs=2)
                ihp = tmpp.tile([128, NLOC], f32, tag="ihp", bufs=2)
                inter = tmpp.tile([128, NLOC], f32, tag="inter", bufs=2)
                ua = tmpp.tile([128, NLOC], f32, tag="ua", bufs=2)
                r = tmpp.tile([128, NLOC], f32, tag="r", bufs=2)
                vd = tmpp.tile([128, NLOC], f32, tag="vd", bufs=2)
                V = outv.tile([128, NLOC], u8, tag="V")

                # iwp = relu(min(bx2,qx2) - max(bx1,qx1)), same for ihp
                nc.vector._custom_dve(iw_op, out=iwp, in0=BX2, in1=BX1,
                                      s0=q(2), s1=q(0))
                nc.vector._custom_dve(iw_op, out=ihp, in0=BY2, in1=BY1,
                                      s0=q(3), s1=q(1))
                nc.vector.scalar_tensor_tensor(inter, iwp, 1.0, ihp,
                                               Alu.mult, Alu.mult)
                nc.vector.scalar_tensor_tensor(ua, inter, -gamma, UA0,
                                               Alu.mult, Alu.add)
                nc.vector.reciprocal_approx_fast(r, ua)
                # vd = 1.0 where invalid (inter<=0), else 0.0
                nc.vector.tensor_scalar(vd, inter, 0.0, None, Alu.is_le)
                # out0 = inter*r - 10*(inter<=0)
                nc.vector._custom_dve(c0_op, out=T[:, :, 0], in0=inter,
                                      in1=r, s0=-10.0)
                nc.scalar.activation(T[:, :, 2], vd, Act.Identity,
                                     bias=q(4), scale=q(5))
                nc.scalar.activation(V, inter, Act.Sign)

                r0 = 128 * t
                ovf = T.rearrange("p f c -> p (f c)")
                cw = 4 * NLOC // OV_CH
                for ci in range(OV_CH):
                    nc.sync.dma_start(
                        out=OV[r0:r0 + 128, ci * cw:(ci + 1) * cw],
                        in_=ovf[:, ci * cw:(ci + 1) * cw])
                if t >= 2:
                    # tensor_index even lanes: per-partition k constant
                    nc.gpsimd.tensor_scalar(IDX[:, :, 0], IDX[:, :, 1],
                                            0.0, q(6), Alu.mult, Alu.add)
                    ixf = IDX.rearrange("p f c -> p (f c)")
                    for ci in range(IX_CH):
                        nc.sync.dma_start(
                            out=IX[r0:r0 + 128, ci * cwi:(ci + 1) * cwi],
                            in_=ixf[:, ci * cwi:(ci + 1) * cwi])
                nc.sync.dma_start(out=VA[r0:r0 + 128, :], in_=V)

    nc.compile()
    return nc


def _split3(x):
    """Exact fp32 -> (hi, mid, lo) bf16 decomposition (hi+mid+lo == x)."""
    import ml_dtypes
    bf = ml_dtypes.bfloat16
    hi = x.astype(bf)
    r1 = x - hi.astype(np.float32)
    mid = r1.astype(bf)
    r2 = r1 - mid.astype(np.float32)
    lo = r2.astype(bf)
    return hi, mid, lo


def _host_prep(boxes, query_boxes, scores_3d, scores_2d, dis_to_lidar_3d,
               criterion):
    boxes = np.ascontiguousarray(boxes, dtype=np.float32)
    qb = np.ascontiguousarray(query_boxes, dtype=np.float32)
    s3d = np.ascontiguousarray(scores_3d, dtype=np.float32).reshape(-1)
    s2d = np.ascontiguousarray(scores_2d, dtype=np.float32).reshape(-1)
    dis = np.ascontiguousarray(dis_to_lidar_3d, dtype=np.float32).reshape(-1)

    barea = (boxes[:, 2] - boxes[:, 0]) * (boxes[:, 3] - boxes[:, 1])
    qarea = (qb[:, 2] - qb[:, 0]) * (qb[:, 3] - qb[:, 1])

    # K-side per-partition scalars, padded to 4*128 rows
    qx1 = np.zeros(KP, np.float32); qy1 = np.zeros(KP, np.float32)
    qx2 = np.ones(KP, np.float32);  qy2 = np.ones(KP, np.float32)
    s2dp = np.zeros(KP, np.float32)
    qap = np.ones(KP, np.float32)
    qx1[:K], qy1[:K], qx2[:K], qy2[:K] = qb[:, 0], qb[:, 1], qb[:, 2], qb[:, 3]
    s2dp[:K] = s2d
    qap[:K] = qarea

    crit = int(criterion)
    # union term: ua = ua0 - gamma*inter, ua0 = alpha*barea + wk
    if crit in (-1, 1):
        wk = qap
    elif crit == 0:
        wk = np.zeros(KP, np.float32)
    else:
        wk = np.ones(KP, np.float32)
    alpha = 1.0 if crit in (-1, 0) else 0.0

    Q = np.zeros((128, 8 * KT), np.float32)
    kk = np.arange(KP, dtype=np.float32)
    for t in range(KT):
        sl = slice(128 * t, 128 * (t + 1))
        Q[:, 8 * t + 0] = qx1[sl]
        Q[:, 8 * t + 1] = qy1[sl]
        Q[:, 8 * t + 2] = qx2[sl]
        Q[:, 8 * t + 3] = qy2[sl]
        Q[:, 8 * t + 4] = s2dp[sl]                        # c2 bias (valid)
        Q[:, 8 * t + 5] = -10.0 - s2dp[sl]                # c2 scale (vd=1)
        Q[:, 8 * t + 6] = kk[sl]                          # tensor_index k
    whi, wmid, wlo = _split3(wk)
    WUA = np.zeros((6, 128 * (KT + 1)), np.dtype(whi.dtype))
    WUA[0:3, :] = 1.0
    WUA[3, :128 * KT] = whi
    WUA[4, :128 * KT] = wmid
    WUA[5, :128 * KT] = wlo
    # last 128-col block: plain-broadcast weights (rows 3-5 zero)
    WUA[3:6, 128 * KT:] = 0.0

    in_maps = []
    for c in range(NCORES):
        s = slice(c * NLOC, (c + 1) * NLOC)
        row = np.zeros(_BC_W, np.float32)
        row[_BC_AREA:_BC_AREA + NLOC] = alpha * barea[s]
        row[_BC_S3D + 0:_BC_S3D + 2 * NLOC:2] = s3d[s]
        row[_BC_S3D + 1:_BC_S3D + 2 * NLOC:2] = dis[s]
        row[_BC_IDX + 1:_BC_IDX + 2 * NLOC:2] = np.arange(
            c * NLOC, (c + 1) * NLOC, dtype=np.float32)
        hi, mid, lo = _split3(row)
        ones = np.ones_like(hi)
        BCc = np.stack([hi, mid, lo, ones, ones, ones], axis=0)
        Ac = np.ascontiguousarray(np.stack(
            [boxes[s, 0], boxes[s, 2], boxes[s, 1], boxes[s, 3]]), np.float32)
        in_maps.append({"A": Ac, "BC": BCc, "WUA": WUA, "Q": Q})
    return in_maps


def run_cores(inputs, trace=False):
    """Compile (cached) + run on 8 cores; returns (results, BassKernelResults)."""
    from concourse.bass_utils import run_bass_kernel_spmd

    crit = int(inputs.get("criterion", -1))
    if crit not in _kernel_cache:
        _kernel_cache[crit] = _build_kernel(crit)
    nc = _kernel_cache[crit]
    in_maps = _host_prep(**inputs)
    res = run_bass_kernel_spmd(nc, in_maps, core_ids=list(range(NCORES)),
                               trace=trace)
    return res.results, res


def kernel(boxes, query_boxes, scores_3d, scores_2d, dis_to_lidar_3d,
           criterion=-1):
    results, _ = run_cores(dict(
        boxes=boxes, query_boxes=query_boxes, scores_3d=scores_3d,
        scores_2d=scores_2d, dis_to_lidar_3d=dis_to_lidar_3d,
        criterion=criterion))

    overlaps = np.empty((K, N, 4), np.float32)
    tensor_index = np.empty((K, N, 2), np.int32)
    valid = np.empty((K, N), bool)
    for c in range(NCORES):
        s = slice(c * NLOC, (c + 1) * NLOC)
        overlaps[:, s, :] = results[c]["OV"].reshape(KP, NLOC, 4)[:K]
        tensor_index[:, s, :] = results[c]["IX"].reshape(KP, NLOC, 2)[:K]
        valid[:, s] = results[c]["VA"].astype(bool)[:K]
    return (overlaps.reshape(K * N, 4), tensor_index.reshape(K * N, 2),
            valid.reshape(K * N))


# revision 35
# speedup vs baseline: 1.1244x; 1.1244x over previous
"""CLOCs pairwise-IoU association kernel for Trainium2 (8 NeuronCores).

Problem: boxes [N=10000,4], query_boxes [K=500,4] -> dense association tensor
  overlaps     [K*N, 4] f32 : (iou|-10, s3d, s2d|-10, dis) per (k,n) pair
  tensor_index [K*N, 2] i32 : (k, n)
  valid        [K*N]   bool : iw>0 & ih>0

Sharding: N split into 8 contiguous strips of 1250 (one per core); every core
holds all K (padded to 512) query boxes on the partition axis, 4 k-tiles of
128 (128-partition DMAs spread across all 16 SDMA engines; narrower ones
collapse onto 4 and serialize).

Broadcast strategy (all exact):
  - box coords + index row: stride-0 partition-broadcast DMA from HBM (SWDGE)
  - (s3d,dis) pair row: TensorE ones[3,128] @ bf16x3(hi/mid/lo) split rows,
    summed in fp32 PSUM -> exact fp32 broadcast at bf16 matmul speed
  - ua0 = box_area[n] + qbox_area[k]: same matmul with the k-term carried in
    three extra lhsT weight rows (bf16x3 split of qarea) against ones rows.
K-side per-box values ride per-partition scalar operands.

Per k-tile (free dim = 1250 n's):
  DVE : t2=max(bx1,qx1); iw=min(bx2,qx2)-t2; t4=max(by1,qy1);
        ih=min(by2,qy2)-t4; inter=max(iw,0)*relu(ih); ua=ua0-inter; r~=1/ua;
        vd=(inter<=0); iou=inter*r; out0=vd*-10+iou
  ACT : relu(ih); out2=scale*vd+bias (c2 select); valid=sign(inter) u8
  PE  : ua0 tile into PSUM
  POOL: tensor_index even lanes = k (off critical path)
The (s3d,dis) output lanes and index odd lanes are constant across k-tiles:
baked once into ping-pong output tiles at setup, only lanes 0/2 rewritten.
Output DMAs are split into column chunks to keep several in flight.
"""

import numpy as np

N = 10000
K = 500
KP = 512                    # K padded to 4*128 partitions
NCORES = 8
NLOC = N // NCORES          # 1250
KT = 4                      # k-tiles of 128
OV_CH = 2                   # overlaps DMA column chunks per k-tile
IX_CH = 1                   # index DMA column chunks per k-tile

# column ranges inside the bf16 broadcast-source array BC [8, 6250]
# rows 0-2: bf16x3 value splits; rows 3-5: 1.0 at AREA cols only (carries the
# qarea k-term); rows 6-7: 1.0 at IDX even cols only (carries the k index)
_BC_AREA = 0
_BC_S3D = NLOC              # 2*NLOC wide, (s3d, dis) interleaved
_BC_IDX = 3 * NLOC          # 2*NLOC wide, (0, n) interleaved
_BC_W = 5 * NLOC

_kernel_cache = {}
_custom_ops = None


def _register_custom_dve_ops():
    """Register two fused DVE ops (runtime-compiled uop programs):
      IW_RELU_ANT : relu(min(Src0, s0) - max(Src1, s1))  -> interval width
      IOU_NEG_ANT : Src0*Src1 + s0*(Src0 <= 0)           -> iou | sentinel
    """
    global _custom_ops
    if _custom_ops is not None:
        return _custom_ops
    import re
    import concourse.dve_ops as dops
    from concourse.dve_spec import Spec, Src0, Src1, C0, C1, Zero, minn, maxx

    def _ref_iw(in0, in1, s0, s1, imm2):
        return np.maximum(
            np.minimum(in0, s0) - np.maximum(in1, s1), 0.0
        ).astype(np.float32)

    def _ref_c0(in0, in1, s0, s1, imm2):
        return (in0 * in1 + s0 * (in0 <= 0.0)).astype(np.float32)

    iw_op = dops.DveOp(
        "IW_RELU_ANT",
        Spec(body=maxx(minn(Src0, C0) - maxx(Src1, C1), Zero),
             reference=_ref_iw),
        subdim=False, uops_sha={})
    c0_op = dops.DveOp(
        "IOU_NEG_ANT",
        Spec(body=Src0 * Src1 + C0 * (Src0 <= Zero),
             reference=_ref_c0),
        subdim=False, uops_sha={})
    next_row = max(dops._SUB_OPCODE_FOR_NAME.values()) + 1
    for op in (iw_op, c0_op):
        dops.OPS.append(op)
        dops.CUSTOM_DVE_SPECS[op.name] = op.spec
        dops._SUB_OPCODE_FOR_NAME[op.name] = next_row
        next_row += 1
        for ver in ("v3",):
            try:
                op.compile(ver)
            except ValueError as e:
                m = re.search(rf"\({ver}: ([0-9a-f]+)", str(e))
                if not m:
                    raise
                op.uops_sha[ver] = m.group(1)
            op.compile(ver)
    _custom_ops = (iw_op, c0_op)
    return _custom_ops


def _build_kernel(criterion: int):
    import concourse.bacc as bacc
    import concourse.tile as tile
    from concourse import mybir

    iw_op, c0_op = _register_custom_dve_ops()

    f32 = mybir.dt.float32
    bf16 = mybir.dt.bfloat16
    i32 = mybir.dt.int32
    u8 = mybir.dt.uint8
    Alu = mybir.AluOpType
    Act = mybir.ActivationFunctionType

    nc = bacc.Bacc("TRN2", target_bir_lowering=False, debug=False,
                   num_devices=NCORES)

    A = nc.dram_tensor("A", [4, NLOC], f32, kind="ExternalInput").ap()
    BC = nc.dram_tensor("BC", [8, _BC_W], bf16, kind="ExternalInput").ap()
    WUA = nc.dram_tensor("WUA", [8, 128 * (KT + 1)], bf16,
                         kind="ExternalInput").ap()
    Q = nc.dram_tensor("Q", [128, 8 * KT], f32, kind="ExternalInput").ap()

    OV = nc.dram_tensor("OV", [KP, 4 * NLOC], f32, kind="ExternalOutput").ap()
    IX = nc.dram_tensor("IX", [KP, 2 * NLOC], i32, kind="ExternalOutput").ap()
    VA = nc.dram_tensor("VA", [KP, NLOC], u8, kind="ExternalOutput").ap()

    # criterion==-1 subtracts inter from the union denominator
    gamma = 1.0 if criterion == -1 else 0.0

    with tile.TileContext(nc) as tc:
        with (
            tc.tile_pool(name="const", bufs=1) as const,
            tc.tile_pool(name="tmp", bufs=1) as tmpp,
            tc.tile_pool(name="outv", bufs=2) as outv,
            tc.tile_pool(name="ps", bufs=2, space="PSUM") as ps,
        ):
            Q_sb = const.tile([128, 8 * KT], f32)
            BC_sb = const.tile([8, _BC_W], bf16)
            WUA_sb = const.tile([8, 128 * (KT + 1)], bf16)
            nc.sync.dma_start(out=Q_sb, in_=Q)
            nc.sync.dma_start(out=BC_sb, in_=BC)
            nc.sync.dma_start(out=WUA_sb, in_=WUA)
            ONES6 = WUA_sb[:, 128 * KT:128 * (KT + 1)]

            BX1 = const.tile([128, NLOC], f32, tag="BX1")
            BX2 = const.tile([128, NLOC], f32, tag="BX2")
            BY1 = const.tile([128, NLOC], f32, tag="BY1")
            BY2 = const.tile([128, NLOC], f32, tag="BY2")
            # coords gate the first DVE op: replicate them via stride-0 DMA
            # (starts immediately, independent of the PE/ACT prologue)
            for row, dst in ((0, BX1), (1, BX2), (2, BY1), (3, BY2)):
                nc.gpsimd.dma_start(
                    out=dst,
                    in_=A[row:row + 1, :].to_broadcast([128, NLOC]))
            Ts = [const.tile([128, NLOC, 4], f32, tag=f"T{i}", name=f"T{i}")
                  for i in range(2)]
            IDXs = [const.tile([128, NLOC, 2], i32, tag=f"IDX{i}",
                               name=f"IDX{i}") for i in range(2)]

            def bcast(col0, width, write, lhsT=None):
                """TensorE-exact broadcast of BC cols [col0, col0+width) to
                128 partitions; `write(psum_chunk, off, cs)` stores it."""
                off = 0
                while off < width:
                    cs = min(512, width - off)
                    pb = ps.tile([128, 512], f32, tag="pb", name="pb")
                    nc.tensor.matmul(pb[:, :cs],
                                     ONES6 if lhsT is None else lhsT,
                                     BC_sb[:, col0 + off:col0 + off + cs],
                                     start=True, stop=True)
                    write(pb[:, :cs], off, cs)
                    off += cs

            def wr_s3d(pbc, off, cs):
                # chunk cols are (s3d,dis) pairs -> T lanes (1,3)
                for T in Ts:
                    nc.scalar.activation(
                        T[:, off // 2:(off + cs) // 2, 1::2],
                        pbc.rearrange("p (f c) -> p f c", c=2), Act.Copy)
            bcast(_BC_S3D, 2 * NLOC, wr_s3d)

            # index tiles 0/1 are fully determined at setup (odd lanes = n
            # from BC rows 0-2, even lanes = k from lhsT rows 6-7): bake and
            # ship them early to fill the DMA-idle prologue window
            cwi = 2 * NLOC // IX_CH
            for t in range(2):
                idx_t = IDXs[t]

                def wr_idx(pbc, off, cs, idx_t=idx_t):
                    nc.scalar.activation(
                        idx_t.rearrange("p f c -> p (f c)")[:, off:off + cs],
                        pbc, Act.Copy)
                bcast(_BC_IDX, 2 * NLOC, wr_idx,
                      lhsT=WUA_sb[:, 128 * t:128 * (t + 1)])
                ixf = idx_t.rearrange("p f c -> p (f c)")
                for ci in range(IX_CH):
                    nc.sync.dma_start(
                        out=IX[128 * t:128 * (t + 1),
                               ci * cwi:(ci + 1) * cwi],
                        in_=ixf[:, ci * cwi:(ci + 1) * cwi])

            # --- steady: one pass per k-tile ---
            for t in range(KT):
                T = Ts[t % 2]
                IDX = IDXs[t % 2]

                def q(j, t=t):
                    return Q_sb[:, 8 * t + j:8 * t + j + 1]

                # ua0 = barea + qarea on TensorE (exact, bf16x3 both sides)
                UA0 = ps.tile([128, NLOC], f32, tag="ua", name="ua0p")
                for c0 in range(0, NLOC, 512):
                    cs = min(512, NLOC - c0)
                    nc.tensor.matmul(UA0[:, c0:c0 + cs],
                                     WUA_sb[:, 128 * t:128 * (t + 1)],
                                     BC_sb[:, _BC_AREA + c0:_BC_AREA + c0 + cs],
                                     start=True, stop=True)

                iwp = tmpp.tile([128, NLOC], f32, tag="iwp", bufs=2)
                ihp = tmpp.tile([128, NLOC], f32, tag="ihp", bufs=2)
                inter = tmpp.tile([128, NLOC], f32, tag="inter", bufs=2)
                ua = tmpp.tile([128, NLOC], f32, tag="ua", bufs=2)
                r = tmpp.tile([128, NLOC], f32, tag="r", bufs=2)
                vd = tmpp.tile([128, NLOC], f32, tag="vd", bufs=2)
                V = outv.tile([128, NLOC], u8, tag="V")

                # iwp = relu(min(bx2,qx2) - max(bx1,qx1)), same for ihp
                nc.vector._custom_dve(iw_op, out=iwp, in0=BX2, in1=BX1,
                                      s0=q(2), s1=q(0))
                nc.vector._custom_dve(iw_op, out=ihp, in0=BY2, in1=BY1,
                                      s0=q(3), s1=q(1))
                nc.vector.scalar_tensor_tensor(inter, iwp, 1.0, ihp,
                                               Alu.mult, Alu.mult)
                nc.vector.scalar_tensor_tensor(ua, inter, -gamma, UA0,
                                               Alu.mult, Alu.add)
                nc.vector.reciprocal_approx_fast(r, ua)
                # vd = 1.0 where invalid (inter<=0), else 0.0
                nc.vector.tensor_scalar(vd, inter, 0.0, None, Alu.is_le)
                # out0 = inter*r - 10*(inter<=0)
                nc.vector._custom_dve(c0_op, out=T[:, :, 0], in0=inter,
                                      in1=r, s0=-10.0)
                nc.scalar.activation(T[:, :, 2], vd, Act.Identity,
                                     bias=q(4), scale=q(5))
                nc.scalar.activation(V, inter, Act.Sign)

                r0 = 128 * t
                ovf = T.rearrange("p f c -> p (f c)")
                cw = 4 * NLOC // OV_CH
                for ci in range(OV_CH):
                    nc.sync.dma_start(
                        out=OV[r0:r0 + 128, ci * cw:(ci + 1) * cw],
                        in_=ovf[:, ci * cw:(ci + 1) * cw])
                if t >= 2:
                    # tensor_index even lanes: per-partition k constant
                    nc.scalar.activation(IDX[:, :, 0], IDX[:, :, 1],
                                         Act.Identity, bias=q(6), scale=0.0)
                    ixf = IDX.rearrange("p f c -> p (f c)")
                    for ci in range(IX_CH):
                        nc.sync.dma_start(
                            out=IX[r0:r0 + 128, ci * cwi:(ci + 1) * cwi],
                            in_=ixf[:, ci * cwi:(ci + 1) * cwi])
                nc.sync.dma_start(out=VA[r0:r0 + 128, :], in_=V)

    nc.compile()
    return nc


def _split3(x):
    """Exact fp32 -> (hi, mid, lo) bf16 decomposition (hi+mid+lo == x)."""
    import ml_dtypes
    bf = ml_dtypes.bfloat16
    hi = x.astype(bf)
    r1 = x - hi.astype(np.float32)
    mid = r1.astype(bf)
    r2 = r1 - mid.astype(np.float32)
    lo = r2.astype(bf)
    return hi, mid, lo


def _host_prep(boxes, query_boxes, scores_3d, scores_2d, dis_to_lidar_3d,
               criterion):
    boxes = np.ascontiguousarray(boxes, dtype=np.float32)
    qb = np.ascontiguousarray(query_boxes, dtype=np.float32)
    s3d = np.ascontiguousarray(scores_3d, dtype=np.float32).reshape(-1)
    s2d = np.ascontiguousarray(scores_2d, dtype=np.float32).reshape(-1)
    dis = np.ascontiguousarray(dis_to_lidar_3d, dtype=np.float32).reshape(-1)

    barea = (boxes[:, 2] - boxes[:, 0]) * (boxes[:, 3] - boxes[:, 1])
    qarea = (qb[:, 2] - qb[:, 0]) * (qb[:, 3] - qb[:, 1])

    # K-side per-partition scalars, padded to 4*128 rows
    qx1 = np.zeros(KP, np.float32); qy1 = np.zeros(KP, np.float32)
    qx2 = np.ones(KP, np.float32);  qy2 = np.ones(KP, np.float32)
    s2dp = np.zeros(KP, np.float32)
    qap = np.ones(KP, np.float32)
    qx1[:K], qy1[:K], qx2[:K], qy2[:K] = qb[:, 0], qb[:, 1], qb[:, 2], qb[:, 3]
    s2dp[:K] = s2d
    qap[:K] = qarea

    crit = int(criterion)
    # union term: ua = ua0 - gamma*inter, ua0 = alpha*barea + wk
    if crit in (-1, 1):
        wk = qap
    elif crit == 0:
        wk = np.zeros(KP, np.float32)
    else:
        wk = np.ones(KP, np.float32)
    alpha = 1.0 if crit in (-1, 0) else 0.0

    Q = np.zeros((128, 8 * KT), np.float32)
    kk = np.arange(KP, dtype=np.float32)
    for t in range(KT):
        sl = slice(128 * t, 128 * (t + 1))
        Q[:, 8 * t + 0] = qx1[sl]
        Q[:, 8 * t + 1] = qy1[sl]
        Q[:, 8 * t + 2] = qx2[sl]
        Q[:, 8 * t + 3] = qy2[sl]
        Q[:, 8 * t + 4] = s2dp[sl]                        # c2 bias (valid)
        Q[:, 8 * t + 5] = -10.0 - s2dp[sl]                # c2 scale (vd=1)
        Q[:, 8 * t + 6] = kk[sl]                          # tensor_index k
    whi, wmid, wlo = _split3(wk)
    WUA = np.zeros((8, 128 * (KT + 1)), np.dtype(whi.dtype))
    WUA[0:3, :] = 1.0
    WUA[3, :128 * KT] = whi
    WUA[4, :128 * KT] = wmid
    WUA[5, :128 * KT] = wlo
    # rows 6-7: exact bf16 int-split of the k index (k = hi + lo)
    WUA[6, :128 * KT] = ((kk.astype(np.int32) // 64) * 64).astype(np.float32)
    WUA[7, :128 * KT] = (kk.astype(np.int32) % 64).astype(np.float32)
    # last 128-col block: plain-broadcast weights (rows 3-7 zero)

    in_maps = []
    for c in range(NCORES):
        s = slice(c * NLOC, (c + 1) * NLOC)
        row = np.zeros(_BC_W, np.float32)
        row[_BC_AREA:_BC_AREA + NLOC] = alpha * barea[s]
        row[_BC_S3D + 0:_BC_S3D + 2 * NLOC:2] = s3d[s]
        row[_BC_S3D + 1:_BC_S3D + 2 * NLOC:2] = dis[s]
        row[_BC_IDX + 1:_BC_IDX + 2 * NLOC:2] = np.arange(
            c * NLOC, (c + 1) * NLOC, dtype=np.float32)
        hi, mid, lo = _split3(row)
        area_ind = np.zeros(_BC_W, hi.dtype)
        area_ind[_BC_AREA:_BC_AREA + NLOC] = 1.0
        idx_ind = np.zeros(_BC_W, hi.dtype)
        idx_ind[_BC_IDX + 0:_BC_IDX + 2 * NLOC:2] = 1.0
        BCc = np.stack([hi, mid, lo, area_ind, area_ind, area_ind,
                        idx_ind, idx_ind], axis=0)
        Ac = np.ascontiguousarray(np.stack(
            [boxes[s, 0], boxes[s, 2], boxes[s, 1], boxes[s, 3]]), np.float32)
        in_maps.append({"A": Ac, "BC": BCc, "WUA": WUA, "Q": Q})
    return in_maps


def run_cores(inputs, trace=False):
    """Compile (cached) + run on 8 cores; returns (results, BassKernelResults)."""
    from concourse.bass_utils import run_bass_kernel_spmd

    crit = int(inputs.get("criterion", -1))
    if crit not in _kernel_cache:
        _kernel_cache[crit] = _build_kernel(crit)
    nc = _kernel_cache[crit]
    in_maps = _host_prep(**inputs)
    res = run_bass_kernel_spmd(nc, in_maps, core_ids=list(range(NCORES)),
                               trace=trace)
    return res.results, res


def kernel(boxes, query_boxes, scores_3d, scores_2d, dis_to_lidar_3d,
           criterion=-1):
    results, _ = run_cores(dict(
        boxes=boxes, query_boxes=query_boxes, scores_3d=scores_3d,
        scores_2d=scores_2d, dis_to_lidar_3d=dis_to_lidar_3d,
        criterion=criterion))

    overlaps = np.empty((K, N, 4), np.float32)
    tensor_index = np.empty((K, N, 2), np.int32)
    valid = np.empty((K, N), bool)
    for c in range(NCORES):
        s = slice(c * NLOC, (c + 1) * NLOC)
        overlaps[:, s, :] = results[c]["OV"].reshape(KP, NLOC, 4)[:K]
        tensor_index[:, s, :] = results[c]["IX"].reshape(KP, NLOC, 2)[:K]
        valid[:, s] = results[c]["VA"].astype(bool)[:K]
    return (overlaps.reshape(K * N, 4), tensor_index.reshape(K * N, 2),
            valid.reshape(K * N))


# revision 36
# speedup vs baseline: 1.1711x; 1.0415x over previous
"""CLOCs pairwise-IoU association kernel for Trainium2 (8 NeuronCores).

Problem: boxes [N=10000,4], query_boxes [K=500,4] -> dense association tensor
  overlaps     [K*N, 4] f32 : (iou|-10, s3d, s2d|-10, dis) per (k,n) pair
  tensor_index [K*N, 2] i32 : (k, n)
  valid        [K*N]   bool : iw>0 & ih>0

Sharding: N split into 8 contiguous strips of 1250 (one per core); every core
holds all K (padded to 512) query boxes on the partition axis, 4 k-tiles of
128 (128-partition DMAs spread across all 16 SDMA engines; narrower ones
collapse onto 4 and serialize).

Broadcast strategy (all exact):
  - box coords + index row: stride-0 partition-broadcast DMA from HBM (SWDGE)
  - (s3d,dis) pair row: TensorE ones[3,128] @ bf16x3(hi/mid/lo) split rows,
    summed in fp32 PSUM -> exact fp32 broadcast at bf16 matmul speed
  - ua0 = box_area[n] + qbox_area[k]: same matmul with the k-term carried in
    three extra lhsT weight rows (bf16x3 split of qarea) against ones rows.
K-side per-box values ride per-partition scalar operands.

Per k-tile (free dim = 1250 n's):
  DVE : t2=max(bx1,qx1); iw=min(bx2,qx2)-t2; t4=max(by1,qy1);
        ih=min(by2,qy2)-t4; inter=max(iw,0)*relu(ih); ua=ua0-inter; r~=1/ua;
        vd=(inter<=0); iou=inter*r; out0=vd*-10+iou
  ACT : relu(ih); out2=scale*vd+bias (c2 select); valid=sign(inter) u8
  PE  : ua0 tile into PSUM
  POOL: tensor_index even lanes = k (off critical path)
The (s3d,dis) output lanes and index odd lanes are constant across k-tiles:
baked once into ping-pong output tiles at setup, only lanes 0/2 rewritten.
Output DMAs are split into column chunks to keep several in flight.
"""

import numpy as np

N = 10000
K = 500
KP = 512                    # K padded to 4*128 partitions
NCORES = 8
NLOC = N // NCORES          # 1250
KT = 4                      # k-tiles of 128
OV_CH = 2                   # overlaps DMA column chunks per k-tile
IX_CH = 1                   # index DMA column chunks per k-tile

# column ranges inside the bf16 broadcast-source array BC [8, 6250]
# rows 0-2: bf16x3 value splits; rows 3-5: 1.0 at AREA cols only (carries the
# qarea k-term); rows 6-7: 1.0 at IDX even cols only (carries the k index)
_BC_AREA = 0
_BC_S3D = NLOC              # 2*NLOC wide, (s3d, dis) interleaved
_BC_IDX = 3 * NLOC          # 2*NLOC wide, (0, n) interleaved
_BC_W = 5 * NLOC

_kernel_cache = {}
_custom_ops = None


def _register_custom_dve_ops():
    """Register two fused DVE ops (runtime-compiled uop programs):
      IW_RELU_ANT : relu(min(Src0, s0) - max(Src1, s1))  -> interval width
      IOU_NEG_ANT : Src0*Src1 + s0*(Src0 <= 0)           -> iou | sentinel
    """
    global _custom_ops
    if _custom_ops is not None:
        return _custom_ops
    import re
    import concourse.dve_ops as dops
    from concourse.dve_spec import Spec, Src0, Src1, C0, C1, Zero, minn, maxx

    def _ref_iw(in0, in1, s0, s1, imm2):
        return np.maximum(
            np.minimum(in0, s0) - np.maximum(in1, s1), 0.0
        ).astype(np.float32)

    def _ref_c0(in0, in1, s0, s1, imm2):
        return (in0 * in1 + s0 * (in0 <= 0.0)).astype(np.float32)

    iw_op = dops.DveOp(
        "IW_RELU_ANT",
        Spec(body=maxx(minn(Src0, C0) - maxx(Src1, C1), Zero),
             reference=_ref_iw),
        subdim=False, uops_sha={})
    c0_op = dops.DveOp(
        "IOU_NEG_ANT",
        Spec(body=Src0 * Src1 + C0 * (Src0 <= Zero),
             reference=_ref_c0),
        subdim=False, uops_sha={})
    next_row = max(dops._SUB_OPCODE_FOR_NAME.values()) + 1
    for op in (iw_op, c0_op):
        dops.OPS.append(op)
        dops.CUSTOM_DVE_SPECS[op.name] = op.spec
        dops._SUB_OPCODE_FOR_NAME[op.name] = next_row
        next_row += 1
        for ver in ("v3",):
            try:
                op.compile(ver)
            except ValueError as e:
                m = re.search(rf"\({ver}: ([0-9a-f]+)", str(e))
                if not m:
                    raise
                op.uops_sha[ver] = m.group(1)
            op.compile(ver)
    _custom_ops = (iw_op, c0_op)
    return _custom_ops


def _build_kernel(criterion: int):
    import concourse.bacc as bacc
    import concourse.tile as tile
    from concourse import mybir

    iw_op, c0_op = _register_custom_dve_ops()

    f32 = mybir.dt.float32
    bf16 = mybir.dt.bfloat16
    i32 = mybir.dt.int32
    u8 = mybir.dt.uint8
    Alu = mybir.AluOpType
    Act = mybir.ActivationFunctionType

    nc = bacc.Bacc("TRN2", target_bir_lowering=False, debug=False,
                   num_devices=NCORES)

    A = nc.dram_tensor("A", [4, NLOC], f32, kind="ExternalInput").ap()
    BC = nc.dram_tensor("BC", [8, _BC_W], bf16, kind="ExternalInput").ap()
    WUA = nc.dram_tensor("WUA", [8, 128 * (KT + 1)], bf16,
                         kind="ExternalInput").ap()
    Q = nc.dram_tensor("Q", [128, 8 * KT], f32, kind="ExternalInput").ap()

    OV = nc.dram_tensor("OV", [KP, 4 * NLOC], f32, kind="ExternalOutput").ap()
    IX = nc.dram_tensor("IX", [KP, 2 * NLOC], i32, kind="ExternalOutput").ap()
    VA = nc.dram_tensor("VA", [KP, NLOC], u8, kind="ExternalOutput").ap()

    # criterion==-1 subtracts inter from the union denominator
    gamma = 1.0 if criterion == -1 else 0.0

    with tile.TileContext(nc) as tc:
        with (
            tc.tile_pool(name="const", bufs=1) as const,
            tc.tile_pool(name="tmp", bufs=1) as tmpp,
            tc.tile_pool(name="outv", bufs=2) as outv,
            tc.tile_pool(name="ps", bufs=2, space="PSUM") as ps,
        ):
            Q_sb = const.tile([128, 8 * KT], f32)
            BC_sb = const.tile([8, _BC_W], bf16)
            WUA_sb = const.tile([8, 128 * (KT + 1)], bf16)
            nc.sync.dma_start(out=Q_sb, in_=Q)
            nc.sync.dma_start(out=BC_sb, in_=BC)
            nc.sync.dma_start(out=WUA_sb, in_=WUA)
            ONES6 = WUA_sb[:, 128 * KT:128 * (KT + 1)]

            BX1 = const.tile([128, NLOC], f32, tag="BX1")
            BX2 = const.tile([128, NLOC], f32, tag="BX2")
            BY1 = const.tile([128, NLOC], f32, tag="BY1")
            BY2 = const.tile([128, NLOC], f32, tag="BY2")
            # coords gate the first DVE op: replicate them via stride-0 DMA
            # (starts immediately, independent of the PE/ACT prologue)
            for row, dst in ((0, BX1), (1, BX2), (2, BY1), (3, BY2)):
                nc.sync.dma_start(
                    out=dst,
                    in_=A[row:row + 1, :].to_broadcast([128, NLOC]))
            Ts = [const.tile([128, NLOC, 4], f32, tag=f"T{i}", name=f"T{i}")
                  for i in range(2)]
            IDXs = [const.tile([128, NLOC, 2], i32, tag=f"IDX{i}",
                               name=f"IDX{i}") for i in range(2)]

            def bcast(col0, width, write, lhsT=None):
                """TensorE-exact broadcast of BC cols [col0, col0+width) to
                128 partitions; `write(psum_chunk, off, cs)` stores it."""
                off = 0
                while off < width:
                    cs = min(512, width - off)
                    pb = ps.tile([128, 512], f32, tag="pb", name="pb")
                    nc.tensor.matmul(pb[:, :cs],
                                     ONES6 if lhsT is None else lhsT,
                                     BC_sb[:, col0 + off:col0 + off + cs],
                                     start=True, stop=True)
                    write(pb[:, :cs], off, cs)
                    off += cs

            def wr_s3d(pbc, off, cs):
                # chunk cols are (s3d,dis) pairs -> T lanes (1,3)
                for T in Ts:
                    nc.scalar.activation(
                        T[:, off // 2:(off + cs) // 2, 1::2],
                        pbc.rearrange("p (f c) -> p f c", c=2), Act.Copy)
            bcast(_BC_S3D, 2 * NLOC, wr_s3d)

            # index tiles 0/1 are fully determined at setup (odd lanes = n
            # from BC rows 0-2, even lanes = k from lhsT rows 6-7): bake and
            # ship them early to fill the DMA-idle prologue window
            cwi = 2 * NLOC // IX_CH
            for t in range(2):
                idx_t = IDXs[t]

                def wr_idx(pbc, off, cs, idx_t=idx_t):
                    nc.scalar.activation(
                        idx_t.rearrange("p f c -> p (f c)")[:, off:off + cs],
                        pbc, Act.Copy)
                bcast(_BC_IDX, 2 * NLOC, wr_idx,
                      lhsT=WUA_sb[:, 128 * t:128 * (t + 1)])
                ixf = idx_t.rearrange("p f c -> p (f c)")
                for ci in range(IX_CH):
                    nc.sync.dma_start(
                        out=IX[128 * t:128 * (t + 1),
                               ci * cwi:(ci + 1) * cwi],
                        in_=ixf[:, ci * cwi:(ci + 1) * cwi])

            # --- steady: one pass per k-tile ---
            for t in range(KT):
                T = Ts[t % 2]
                IDX = IDXs[t % 2]

                def q(j, t=t):
                    return Q_sb[:, 8 * t + j:8 * t + j + 1]

                # ua0 = barea + qarea on TensorE (exact, bf16x3 both sides)
                UA0 = ps.tile([128, NLOC], f32, tag="ua", name="ua0p")
                for c0 in range(0, NLOC, 512):
                    cs = min(512, NLOC - c0)
                    nc.tensor.matmul(UA0[:, c0:c0 + cs],
                                     WUA_sb[:, 128 * t:128 * (t + 1)],
                                     BC_sb[:, _BC_AREA + c0:_BC_AREA + c0 + cs],
                                     start=True, stop=True)

                iwp = tmpp.tile([128, NLOC], f32, tag="iwp", bufs=2)
                ihp = tmpp.tile([128, NLOC], f32, tag="ihp", bufs=2)
                inter = tmpp.tile([128, NLOC], f32, tag="inter", bufs=2)
                ua = tmpp.tile([128, NLOC], f32, tag="ua", bufs=2)
                r = tmpp.tile([128, NLOC], f32, tag="r", bufs=2)
                vd = tmpp.tile([128, NLOC], f32, tag="vd", bufs=2)
                V = outv.tile([128, NLOC], u8, tag="V")

                # iwp = relu(min(bx2,qx2) - max(bx1,qx1)), same for ihp
                nc.vector._custom_dve(iw_op, out=iwp, in0=BX2, in1=BX1,
                                      s0=q(2), s1=q(0))
                nc.vector._custom_dve(iw_op, out=ihp, in0=BY2, in1=BY1,
                                      s0=q(3), s1=q(1))
                nc.vector.scalar_tensor_tensor(inter, iwp, 1.0, ihp,
                                               Alu.mult, Alu.mult)
                nc.vector.scalar_tensor_tensor(ua, inter, -gamma, UA0,
                                               Alu.mult, Alu.add)
                nc.vector.reciprocal_approx_fast(r, ua)
                # vd = 1.0 where invalid (inter<=0), else 0.0
                nc.vector.tensor_scalar(vd, inter, 0.0, None, Alu.is_le)
                # out0 = inter*r - 10*(inter<=0); last tile in halves so its
                # output DMAs start before the full tile finishes
                r0 = 128 * t
                ovf = T.rearrange("p f c -> p (f c)")
                cw = 4 * NLOC // OV_CH
                nh = OV_CH if t == KT - 1 else 1
                hw = NLOC // nh
                for h in range(nh):
                    hs = slice(h * hw, (h + 1) * hw)
                    nc.vector._custom_dve(c0_op, out=T[:, hs, 0],
                                          in0=inter[:, hs], in1=r[:, hs],
                                          s0=-10.0)
                    nc.scalar.activation(T[:, hs, 2], vd[:, hs], Act.Identity,
                                         bias=q(4), scale=q(5))
                    if nh == OV_CH:
                        nc.sync.dma_start(
                            out=OV[r0:r0 + 128, h * cw:(h + 1) * cw],
                            in_=ovf[:, h * cw:(h + 1) * cw])
                if nh != OV_CH:
                    for ci in range(OV_CH):
                        nc.sync.dma_start(
                            out=OV[r0:r0 + 128, ci * cw:(ci + 1) * cw],
                            in_=ovf[:, ci * cw:(ci + 1) * cw])
                nc.scalar.activation(V, inter, Act.Sign)
                if t >= 2:
                    # tensor_index even lanes: per-partition k constant
                    nc.scalar.activation(IDX[:, :, 0], IDX[:, :, 1],
                                         Act.Identity, bias=q(6), scale=0.0)
                    ixf = IDX.rearrange("p f c -> p (f c)")
                    for ci in range(IX_CH):
                        nc.sync.dma_start(
                            out=IX[r0:r0 + 128, ci * cwi:(ci + 1) * cwi],
                            in_=ixf[:, ci * cwi:(ci + 1) * cwi])
                nc.sync.dma_start(out=VA[r0:r0 + 128, :], in_=V)

    nc.compile()
    return nc


def _split3(x):
    """Exact fp32 -> (hi, mid, lo) bf16 decomposition (hi+mid+lo == x)."""
    import ml_dtypes
    bf = ml_dtypes.bfloat16
    hi = x.astype(bf)
    r1 = x - hi.astype(np.float32)
    mid = r1.astype(bf)
    r2 = r1 - mid.astype(np.float32)
    lo = r2.astype(bf)
    return hi, mid, lo


def _host_prep(boxes, query_boxes, scores_3d, scores_2d, dis_to_lidar_3d,
               criterion):
    boxes = np.ascontiguousarray(boxes, dtype=np.float32)
    qb = np.ascontiguousarray(query_boxes, dtype=np.float32)
    s3d = np.ascontiguousarray(scores_3d, dtype=np.float32).reshape(-1)
    s2d = np.ascontiguousarray(scores_2d, dtype=np.float32).reshape(-1)
    dis = np.ascontiguousarray(dis_to_lidar_3d, dtype=np.float32).reshape(-1)

    barea = (boxes[:, 2] - boxes[:, 0]) * (boxes[:, 3] - boxes[:, 1])
    qarea = (qb[:, 2] - qb[:, 0]) * (qb[:, 3] - qb[:, 1])

    # K-side per-partition scalars, padded to 4*128 rows
    qx1 = np.zeros(KP, np.float32); qy1 = np.zeros(KP, np.float32)
    qx2 = np.ones(KP, np.float32);  qy2 = np.ones(KP, np.float32)
    s2dp = np.zeros(KP, np.float32)
    qap = np.ones(KP, np.float32)
    qx1[:K], qy1[:K], qx2[:K], qy2[:K] = qb[:, 0], qb[:, 1], qb[:, 2], qb[:, 3]
    s2dp[:K] = s2d
    qap[:K] = qarea

    crit = int(criterion)
    # union term: ua = ua0 - gamma*inter, ua0 = alpha*barea + wk
    if crit in (-1, 1):
        wk = qap
    elif crit == 0:
        wk = np.zeros(KP, np.float32)
    else:
        wk = np.ones(KP, np.float32)
    alpha = 1.0 if crit in (-1, 0) else 0.0

    Q = np.zeros((128, 8 * KT), np.float32)
    kk = np.arange(KP, dtype=np.float32)
    for t in range(KT):
        sl = slice(128 * t, 128 * (t + 1))
        Q[:, 8 * t + 0] = qx1[sl]
        Q[:, 8 * t + 1] = qy1[sl]
        Q[:, 8 * t + 2] = qx2[sl]
        Q[:, 8 * t + 3] = qy2[sl]
        Q[:, 8 * t + 4] = s2dp[sl]                        # c2 bias (valid)
        Q[:, 8 * t + 5] = -10.0 - s2dp[sl]                # c2 scale (vd=1)
        Q[:, 8 * t + 6] = kk[sl]                          # tensor_index k
    whi, wmid, wlo = _split3(wk)
    WUA = np.zeros((8, 128 * (KT + 1)), np.dtype(whi.dtype))
    WUA[0:3, :] = 1.0
    WUA[3, :128 * KT] = whi
    WUA[4, :128 * KT] = wmid
    WUA[5, :128 * KT] = wlo
    # rows 6-7: exact bf16 int-split of the k index (k = hi + lo)
    WUA[6, :128 * KT] = ((kk.astype(np.int32) // 64) * 64).astype(np.float32)
    WUA[7, :128 * KT] = (kk.astype(np.int32) % 64).astype(np.float32)
    # last 128-col block: plain-broadcast weights (rows 3-7 zero)

    in_maps = []
    for c in range(NCORES):
        s = slice(c * NLOC, (c + 1) * NLOC)
        row = np.zeros(_BC_W, np.float32)
        row[_BC_AREA:_BC_AREA + NLOC] = alpha * barea[s]
        row[_BC_S3D + 0:_BC_S3D + 2 * NLOC:2] = s3d[s]
        row[_BC_S3D + 1:_BC_S3D + 2 * NLOC:2] = dis[s]
        row[_BC_IDX + 1:_BC_IDX + 2 * NLOC:2] = np.arange(
            c * NLOC, (c + 1) * NLOC, dtype=np.float32)
        hi, mid, lo = _split3(row)
        area_ind = np.zeros(_BC_W, hi.dtype)
        area_ind[_BC_AREA:_BC_AREA + NLOC] = 1.0
        idx_ind = np.zeros(_BC_W, hi.dtype)
        idx_ind[_BC_IDX + 0:_BC_IDX + 2 * NLOC:2] = 1.0
        BCc = np.stack([hi, mid, lo, area_ind, area_ind, area_ind,
                        idx_ind, idx_ind], axis=0)
        Ac = np.ascontiguousarray(np.stack(
            [boxes[s, 0], boxes[s, 2], boxes[s, 1], boxes[s, 3]]), np.float32)
        in_maps.append({"A": Ac, "BC": BCc, "WUA": WUA, "Q": Q})
    return in_maps


def run_cores(inputs, trace=False):
    """Compile (cached) + run on 8 cores; returns (results, BassKernelResults)."""
    from concourse.bass_utils import run_bass_kernel_spmd

    crit = int(inputs.get("criterion", -1))
    if crit not in _kernel_cache:
        _kernel_cache[crit] = _build_kernel(crit)
    nc = _kernel_cache[crit]
    in_maps = _host_prep(**inputs)
    res = run_bass_kernel_spmd(nc, in_maps, core_ids=list(range(NCORES)),
                               trace=trace)
    return res.results, res


def kernel(boxes, query_boxes, scores_3d, scores_2d, dis_to_lidar_3d,
           criterion=-1):
    results, _ = run_cores(dict(
        boxes=boxes, query_boxes=query_boxes, scores_3d=scores_3d,
        scores_2d=scores_2d, dis_to_lidar_3d=dis_to_lidar_3d,
        criterion=criterion))

    overlaps = np.empty((K, N, 4), np.float32)
    tensor_index = np.empty((K, N, 2), np.int32)
    valid = np.empty((K, N), bool)
    for c in range(NCORES):
        s = slice(c * NLOC, (c + 1) * NLOC)
        overlaps[:, s, :] = results[c]["OV"].reshape(KP, NLOC, 4)[:K]
        tensor_index[:, s, :] = results[c]["IX"].reshape(KP, NLOC, 2)[:K]
        valid[:, s] = results[c]["VA"].astype(bool)[:K]
    return (overlaps.reshape(K * N, 4), tensor_index.reshape(K * N, 2),
            valid.reshape(K * N))
